# revision 50
# baseline (speedup 1.0000x reference)
"""Multi-head attention (B=2, S=2048, D=1024, H=16) on 8 trn2 NeuronCores.

Sharding: core c owns heads {2c, 2c+1} for both batches (tensor parallel by
head). Token axis is flattened b-major: T = B*S = 4096. Each core computes a
full-size partial output (its 128 rows of concat x its Wo row-block); the
host sums the 8 partials (the unshard for row-sharded Wo).

Single fused pipeline per core (no phase barriers). All inputs stream in as
[D, 512]-token slabs; K/Q/V project per-slab, and attention for batch 0
starts ~13us in while batch 1 is still loading. Per 512-token query block:
 - Scores in transposed orientation [j, i]: both heads share one 2-bank PSUM
   tile, one fused exp per j-step on ACT (scale=1/8 folded in), causal
   masking via one fused affine_select on Pool.
 - PV reoriented: the exp tile is the stationary operand, V (+ones column)
   moves -> out [i, 65] per 128-row i-subtile. Costs 65 cols per live (j,i)
   pair instead of up to 512, and the softmax denominator lands in column
   64, making normalization a per-partition scalar multiply (no partition
   broadcast). PSUM allows one open accumulation group per bank, so the
   whole PV runs subtile-sequentially as a deferred burst hooked into the
   NEXT block's j-loop (exp tiles are retained in SBUF).
 - Normalized [i, d] tiles are PE-transposed (identity matmul) back into
   CT's [d, i]; the output projection trails a block behind, spread across
   the following blocks' j-loops in token-tile units, stored via gpsimd DMA.
 - Emission order is hand-scheduled with per-j-step hooks so DMA-gated work
   never head-of-line blocks an engine stream.
Engines: SP=input DMA, PE=matmuls, ACT=exp (+tail copies), DVE=normalize +
PSUM->SBUF copies, Pool=causal masks + output stores. All matmuls bf16 with
fp32 PSUM accumulation; host pre-transposes/casts and sums the partials.
"""

import sys

sys.path.insert(0, "/opt/trn_rl_repo")

import numpy as np
import ml_dtypes

import concourse.bass as bass
import concourse.mybir as mybir
import concourse.tile as tile
from concourse import bacc
from concourse import bass_utils

B, S, D, H = 2, 2048, 1024, 16
DK = D // H              # 64
N_CORES = 8
HPC = H // N_CORES       # heads per core (2)
EPC = HPC * DK           # 128 projected cols per core
T = B * S                # 4096 flattened tokens
TB = S                   # tokens per batch
IT = 512                 # i (query) tile
JT = 128                 # j (key) tile
NIT = TB // IT           # 4 i-tiles per batch
NJT = TB // JT           # 16 j-tiles per batch
ND = D // 128            # 8 contraction tiles
VST = DK + 1             # 65: V block width with ones column
NJ_ALL = T // JT         # 32 j-tiles across both batches
HB = TB // 2             # 1024: half-batch token count

bf16 = mybir.dt.bfloat16
f32 = mybir.dt.float32
f16 = mybir.dt.float16
BF = ml_dtypes.bfloat16

_CACHE: dict = {}


def _build(mode: str, repeats: int = 1, upto: str = "full"):
    """mode: 'causal' | 'none' | 'generic'. repeats>1 builds a timing variant
    executing the whole body N times in one NEFF. upto: 'full' | 'p1'
    truncates after projections (timing ablation only)."""
    nc = bacc.Bacc("TRN2", target_bir_lowering=False, debug=False,
                   enable_asserts=False, num_devices=N_CORES)

    xq = nc.dram_tensor("xq", [D, T], bf16, kind="ExternalInput")
    xk = nc.dram_tensor("xk", [D, T], bf16, kind="ExternalInput")
    xv = nc.dram_tensor("xv", [D, T], bf16, kind="ExternalInput")
    w3 = nc.dram_tensor("w3", [D, 3 * EPC], bf16, kind="ExternalInput")
    wo = nc.dram_tensor("wo", [128, D], bf16, kind="ExternalInput")
    if mode == "generic":
        bias = nc.dram_tensor("bias", [S, S], bf16, kind="ExternalInput")
    out = nc.dram_tensor("out", [T, D], f16, kind="ExternalOutput")

    Exp = mybir.ActivationFunctionType.Exp

    with tile.TileContext(nc) as tc:
      for _rep in range(repeats):
        with (
            tc.tile_pool(name="consts", bufs=1) as consts,
            tc.tile_pool(name="persist", bufs=1) as persist,
        ):
            wsb = consts.tile([128, ND, 3 * EPC], bf16, tag="wsb", name="wsb")
            wob = consts.tile([128, D], bf16, tag="wob", name="wob")
            ident = consts.tile([128, 128], bf16, tag="ident", name="ident")

            QT = persist.tile([128, T], bf16, tag="QT")
            KT = persist.tile([128, T], bf16, tag="KT")
            CT = persist.tile([128, T], bf16, tag="CT")
            V_all = persist.tile([128, NJ_ALL * HPC * VST], bf16, tag="V_all")
            v4 = V_all[:].rearrange("p (t h c) -> p (t h) c",
                                    t=NJ_ALL, h=HPC, c=VST)
            nc.vector.memset(v4[:, :, DK:DK + 1], 1.0)
            # identity for PE transposes: ones where col == partition
            nc.gpsimd.memset(ident[:], 1.0)
            nc.gpsimd.affine_select(
                out=ident[:], in_=ident[:],
                compare_op=mybir.AluOpType.is_equal, fill=0.0,
                base=0, pattern=[[1, 128]], channel_multiplier=-1)

            with (
                tc.tile_pool(name="xpool", bufs=8) as xpool,
                tc.tile_pool(name="psT", bufs=1, space="PSUM") as psT,
                tc.tile_pool(name="psO", bufs=1, space="PSUM") as psO,
                tc.tile_pool(name="psM", bufs=1, space="PSUM") as psM,
                tc.tile_pool(name="sbE", bufs=1) as sbE,
                tc.tile_pool(name="sbR", bufs=1) as sbR,
                tc.tile_pool(name="sbF", bufs=1) as sbF,
                tc.tile_pool(name="biasp", bufs=4) as biasp,
            ):
                # ---------------- DMA issue (SP), pipeline order ----------
                # wq|wk slice first: it gates the first K/Q projections
                nc.sync.dma_start(
                    wsb[:, :, 0:2 * EPC],
                    w3.ap()[:, 0:2 * EPC].rearrange("(d p) e -> p d e",
                                                    p=128))

                slabs = {}   # (key, b, sl) -> tile [128, ND, 512]

                def load_slab(key, b, sl, halves=1):
                    xd = {"k": xk, "q": xq, "v": xv}[key]
                    t0 = TB * b + 512 * sl
                    xt = xpool.tile([128, ND, 512], bf16, tag="xs",
                                    name=f"xs_{key}{b}{sl}")
                    hd = ND // halves
                    for h in range(halves):
                        nc.sync.dma_start(
                            xt[:, hd * h:hd * (h + 1), :],
                            xd.ap()[128 * hd * h:128 * hd * (h + 1),
                                    t0:t0 + 512]
                            .rearrange("(d p) t -> p d t", p=128))
                    slabs[key, b, sl] = xt

                # order tuned for the pipeline: K/Q slabs front-run V
                # slabs (PV work is deferred a block behind, so V has slack)
                load_slab("k", 0, 0, halves=2)
                load_slab("q", 0, 0, halves=2)
                nc.sync.dma_start(
                    wsb[:, :, 2 * EPC:3 * EPC],
                    w3.ap()[:, 2 * EPC:3 * EPC]
                    .rearrange("(d p) e -> p d e", p=128))
                nc.sync.dma_start(wob[:], wo.ap())
                for key, sl in (("k", 1), ("q", 1), ("v", 0), ("k", 2),
                                ("q", 2), ("v", 1), ("k", 3), ("q", 3),
                                ("v", 2), ("v", 3)):
                    load_slab(key, 0, sl)
                for key, sl in (("k", 0), ("q", 0), ("k", 1), ("q", 1),
                                ("v", 0), ("k", 2), ("q", 2), ("v", 1),
                                ("k", 3), ("q", 3), ("v", 2), ("v", 3)):
                    load_slab(key, 1, sl)

                # ---------------- emission helpers -----------------------
                WOFF = {"q": 0, "k": EPC, "v": 2 * EPC}

                def proj_kq(key, b, st):
                    """Project one 512-token slab of K or Q; copy to SBUF on
                    DVE (ACT's exp chain paces attention; keep it clear)."""
                    dest = KT if key == "k" else QT
                    t0 = TB * b + IT * st
                    xt = slabs[key, b, st]
                    ps = psM.tile([128, IT], f32, tag="mm", bufs=2,
                                  name=f"mm_{key}{b}{st}")
                    for d in range(ND):
                        nc.tensor.matmul(
                            ps[:],
                            wsb[:, d, WOFF[key]:WOFF[key] + EPC],
                            xt[:, d, :],
                            start=(d == 0), stop=(d == ND - 1))
                    nc.vector.tensor_copy(dest[:, t0:t0 + IT], ps[:])

                def proj_v(b, sl):
                    """Project one 512-token V slab into V_all (4 j-tiles)."""
                    vt = slabs["v", b, sl]
                    ps = psM.tile([128, 512], f32, tag="mm", bufs=2,
                                  name=f"mmv{b}{sl}")
                    # jf-outer: PSUM allows only one open accumulation
                    # group per bank (start marks the whole bank pending-zero)
                    for jf in range(4):
                        for d in range(ND):
                            nc.tensor.matmul(
                                ps[:, 128 * jf:128 * (jf + 1)],
                                vt[:, d, 128 * jf:128 * (jf + 1)],
                                wsb[:, d, 2 * EPC:3 * EPC],
                                start=(d == 0), stop=(d == ND - 1))
                    jt0 = NJT * b + 4 * sl
                    # gpsimd cannot read PSUM on HW; DVE does the scatter
                    nc.vector.tensor_copy(
                        v4[:, HPC * jt0:HPC * (jt0 + 4), 0:DK],
                        ps[:].rearrange("p (j h c) -> p (j h) c",
                                        j=4, h=HPC, c=DK))

                def emit_final_parts(b, it, split=False):
                    """Output projection for i-block (b, it) as 5 thunks:
                    4 token-tile units (2 pf matmuls + DVE/Pool copies each)
                    plus a store thunk — spread across the hosting block's
                    j-loop so pf PSUM-slot waits overlap scores work.
                    split=True: per-tt stores (pipeline drain)."""
                    i0 = TB * b + IT * it
                    state = {}

                    def unit(tt):
                        def run():
                            if "of" not in state:
                                state["of"] = sbF.tile(
                                    [128, IT // 128, D], f16, tag="of",
                                    bufs=2, name="of")
                            of = state["of"]
                            t0 = i0 + 128 * tt
                            for eh in range(2):
                                if split and eh == 1:
                                    pf = psO.tile([128, 512], f32, tag="po",
                                                  bufs=2, name="pf")
                                else:
                                    pf = psM.tile([128, 512], f32, tag="mm",
                                                  bufs=2, name="pf")
                                nc.tensor.matmul(
                                    pf[:], CT[:, t0:t0 + 128],
                                    wob[:, 512 * eh:512 * (eh + 1)],
                                    start=True, stop=True)
                                dst = of[:, tt, 512 * eh:512 * (eh + 1)]
                                if split and eh == 1:
                                    nc.scalar.copy(dst, pf[:])
                                else:
                                    nc.vector.tensor_copy(dst, pf[:])
                            if split:
                                # SP is idle during the drain; its HWDGE path
                                # issues faster than Pool's SWDGE
                                nc.sync.dma_start(
                                    out.ap()[t0:t0 + 128, :], of[:, tt, :])
                        return run

                    def store():
                        nc.gpsimd.dma_start(
                            out.ap()[i0:i0 + IT, :].rearrange(
                                "(tt p) e -> p tt e", p=128),
                            state["of"])

                    thunks = [unit(tt) for tt in range(IT // 128)]
                    if not split:
                        thunks.append(store)
                    return thunks

                def emit_final(b, it, split=False):
                    for fn in emit_final_parts(b, it, split):
                        fn()

                def attention_block(b, it, pre=(), mid=None,
                                    expose=None):
                    """One i-block of attention: scores+exp j-loop only. PV
                    (reoriented) + normalize + transpose into CT are wrapped
                    in the returned `finish` closure, which the caller places
                    as a hook in the NEXT block (po bufs=2 covers one block
                    of deferral). PV runs subtile-outer: PSUM permits a
                    single open accumulation group per bank, so each [i,65]
                    region completes its j-accumulation before the next
                    starts. `pre` thunks emit first; `mid[jl]` thunks emit
                    right after j-step jl."""
                    mid = mid or {}
                    for fn in pre:
                        fn()
                    il0 = IT * it
                    i0 = TB * b + il0
                    njt = (il0 + IT) // JT if mode == "causal" else NJT
                    pos = [psO.tile([128, 512], f32, tag="po", bufs=2,
                                    name=f"po{hl}") for hl in range(HPC)]
                    exs = []

                    def pv_region(s):
                        for hl in range(HPC):
                            jmax = njt - 1
                            if mode == "causal":
                                jmax = min(jmax, 4 * it + s)
                            for jl in range(jmax + 1):
                                ex, off = exs[jl]
                                jabs = NJT * b + jl
                                voff = VST * (HPC * jabs + hl)
                                c0 = 128 * s - off
                                nc.tensor.matmul(
                                    pos[hl][:, 65 * s:65 * s + 65],
                                    ex[:, hl, c0:c0 + 128],
                                    V_all[:, voff:voff + VST],
                                    start=(jl == 0), stop=(jl == jmax))

                    rec = sbR.tile([128, HPC, 4], f32, tag="rec", bufs=2,
                                   name="rec")
                    P = sbR.tile([128, 4, 128], bf16, tag="Pst", bufs=2,
                                 name="P")
                    tp = psO.tile([128, 512], bf16, tag="po", bufs=2,
                                  name="tp")

                    if expose is not None:
                        expose["pv"] = pv_region

                    for jl in range(njt):
                        jabs = NJT * b + jl
                        j0 = JT * jl
                        diag = mode == "causal" and j0 > il0 - 1
                        off = max(0, j0 - il0) if mode == "causal" else 0
                        nl = IT - off
                        pt = psT.tile([128, 2, IT], f32, tag="pt", bufs=2,
                                      name="pt")
                        bs = None
                        if mode == "generic":
                            bs = biasp.tile([128, IT], bf16, tag="bias",
                                            name="bs")
                            nc.sync.dma_start(
                                bs[:],
                                bias.ap()[JT * jl:JT * (jl + 1),
                                          il0:il0 + IT])
                        for hl in range(HPC):
                            pb = 64 * hl
                            nc.tensor.matmul(
                                pt[:, hl, 0:nl],
                                KT[pb:pb + DK, JT * jabs:JT * (jabs + 1)],
                                QT[pb:pb + DK, i0 + off:i0 + IT],
                                start=True, stop=True)
                            if bs is not None:
                                nc.vector.tensor_add(pt[:, hl, 0:nl],
                                                     pt[:, hl, 0:nl],
                                                     bs[:, off:IT])
                        ex = sbE.tile([128, 2, IT], bf16, tag="expp",
                                      bufs=18, name="ex")
                        nc.scalar.activation(ex[:, :, 0:nl], pt[:, :, 0:nl],
                                             Exp, scale=0.125)
                        if diag:
                            # triangular part is the first JT live cols of
                            # both heads: keep iff col_in_tile >= partition
                            nc.gpsimd.affine_select(
                                out=ex[:, :, 0:JT], in_=ex[:, :, 0:JT],
                                compare_op=mybir.AluOpType.is_ge,
                                fill=0.0, base=0,
                                pattern=[[0, 2], [1, JT]],
                                channel_multiplier=-1)
                        exs.append((ex, off))
                        for fn in mid.get(jl, ()):
                            fn()

                    def norm_mul(hl, s, on_act):
                        dst = P[:, s, 64 * hl:64 * (hl + 1)]
                        srcp = pos[hl][:, 65 * s:65 * s + DK]
                        rc = rec[:, hl, s:s + 1]
                        if on_act:
                            nc.scalar.mul(dst, srcp, rc)
                        else:
                            nc.vector.tensor_scalar(
                                dst, srcp, rc, None, mybir.AluOpType.mult)

                    def finish(first_s=0, tail=False):
                        # normalize: per-partition reciprocal of denominator
                        # columns, scale the 64 data cols of each subtile;
                        # then PE-transpose [i, d] back into CT's [d, i].
                        # tail=True (pipeline drain): emit the last PV region
                        # first so DVE/ACT norm work for the finished regions
                        # overlaps it, and split muls across DVE and ACT.
                        for s in range(first_s, 4):
                            pv_region(s)
                        if tail:
                            for hl in range(HPC):
                                nc.vector.reciprocal(
                                    rec[:, hl, 0:3],
                                    pos[hl][:, DK:DK + 3 * VST:VST])
                            for s in range(3):
                                for hl in range(HPC):
                                    norm_mul(hl, s, on_act=(hl == 1))
                            for hl in range(HPC):
                                nc.vector.reciprocal(
                                    rec[:, hl, 3:4],
                                    pos[hl][:, DK + 3 * VST:
                                            DK + 3 * VST + 1])
                                norm_mul(hl, 3, on_act=(hl == 1))
                        else:
                            for hl in range(HPC):
                                nc.vector.reciprocal(
                                    rec[:, hl, :],
                                    pos[hl][:, DK:DK + 4 * VST:VST])
                            for hl in range(HPC):
                                for s in range(4):
                                    norm_mul(hl, s, on_act=False)
                        for s in range(4):
                            nc.tensor.transpose(
                                tp[:, 128 * s:128 * (s + 1)], P[:, s, :],
                                ident[:])
                        if tail:
                            # two-way CT copy: the first drain outproj tile
                            # (and the serial store queue behind it) gates
                            # only on the s0 slice
                            nc.vector.tensor_copy(CT[:, i0:i0 + 128],
                                                  tp[:, 0:128])
                            nc.vector.tensor_copy(CT[:, i0 + 128:i0 + IT],
                                                  tp[:, 128:])
                        else:
                            nc.vector.tensor_copy(CT[:, i0:i0 + IT], tp[:])

                    return finish

                # ---------------- pipeline ------------------------------
                proj_kq("k", 0, 0)
                proj_kq("q", 0, 0)

                if upto == "p1":
                    # ablation: projections only
                    for key in ("k", "q"):
                        for b in range(B):
                            for st in range(4):
                                if (key, b, st) != ("k", 0, 0) and \
                                        (key, b, st) != ("q", 0, 0):
                                    proj_kq(key, b, st)
                    for b in range(B):
                        for sl in range(4):
                            proj_v(b, sl)
                    with tc.tile_pool(name="junk", bufs=1) as jp:
                        jt_ = jp.tile([128, D], f16, name="junk")
                        nc.vector.memset(jt_[:], 0.0)
                        for ttj in range(T // 128):
                            nc.sync.dma_start(
                                out.ap()[128 * ttj:128 * (ttj + 1), :],
                                jt_[:])
                    continue

                fin = {}

                def fin_run(b, it):
                    return lambda: fin[b, it]()

                def mids(*pairs):
                    d = {}
                    for jl, fn in pairs:
                        d.setdefault(jl, []).append(fn)
                    return d

                ef = {}
                fin[0, 0] = attention_block(
                    0, 0,
                    mid=mids((0, lambda: proj_kq("k", 0, 1)),
                             (3, lambda: proj_kq("q", 0, 1))))
                fin[0, 1] = attention_block(
                    0, 1,
                    mid=mids((3, lambda: proj_v(0, 0)),
                             (4, fin_run(0, 0)),
                             (5, lambda: proj_kq("k", 0, 2)),
                             (7, lambda: proj_kq("q", 0, 2))))
                ef[0, 0] = emit_final_parts(0, 0)
                fin[0, 2] = attention_block(
                    0, 2,
                    mid=mids((1, ef[0, 0][0]), (2, lambda: proj_v(0, 1)),
                             (3, fin_run(0, 1)), (4, ef[0, 0][1]),
                             (5, lambda: proj_kq("k", 0, 3)),
                             (6, ef[0, 0][2]),
                             (8, lambda: proj_kq("q", 0, 3)),
                             (9, ef[0, 0][3]), (10, ef[0, 0][4])))
                ef[0, 1] = emit_final_parts(0, 1)
                fin[0, 3] = attention_block(
                    0, 3,
                    mid=mids((0, lambda: proj_v(0, 2)),
                             (1, fin_run(0, 2)),
                             (2, ef[0, 1][0]),
                             (3, lambda: proj_v(0, 3)),
                             (5, ef[0, 1][1]),
                             (6, lambda: proj_kq("k", 1, 0)),
                             (8, ef[0, 1][2]),
                             (10, lambda: proj_kq("q", 1, 0)),
                             (11, ef[0, 1][3]), (12, ef[0, 1][4])))
                ef[0, 2] = emit_final_parts(0, 2)
                fin[1, 0] = attention_block(
                    1, 0,
                    mid=mids((0, lambda: proj_kq("k", 1, 1)),
                             (1, ef[0, 2][0]),
                             (1, lambda: proj_kq("q", 1, 1)),
                             (2, fin_run(0, 3)),
                             (3, ef[0, 2][1])))
                fin[1, 1] = attention_block(
                    1, 1,
                    mid=mids((0, ef[0, 2][2]),
                             (1, lambda: proj_v(1, 0)),
                             (2, fin_run(1, 0)),
                             (3, lambda: proj_kq("k", 1, 2)),
                             (4, ef[0, 2][3]), (5, ef[0, 2][4]),
                             (5, lambda: proj_kq("q", 1, 2))))
                ef[0, 3] = emit_final_parts(0, 3)
                fin[1, 2] = attention_block(
                    1, 2,
                    mid=mids((0, ef[0, 3][0]),
                             (1, lambda: proj_v(1, 1)),
                             (2, fin_run(1, 1)),
                             (3, ef[0, 3][1]),
                             (4, lambda: proj_kq("k", 1, 3)),
                             (5, ef[0, 3][2]),
                             (7, lambda: proj_kq("q", 1, 3)),
                             (8, ef[0, 3][3]), (9, ef[0, 3][4])))
                ef[1, 0] = emit_final_parts(1, 0)
                ef[1, 1] = emit_final_parts(1, 1)
                ef[1, 2] = emit_final_parts(1, 2, split=True)
                pv13 = {}
                fin[1, 3] = attention_block(
                    1, 3,
                    mid=mids((0, lambda: proj_v(1, 2)),
                             (1, fin_run(1, 2)),
                             (2, ef[1, 2][0]),
                             (3, lambda: proj_v(1, 3)),
                             (4, ef[1, 2][1]), (5, ef[1, 2][2]),
                             (6, ef[1, 2][3]),
                             (7, ef[1, 0][0]), (8, ef[1, 0][1]),
                             (9, ef[1, 0][2]), (10, ef[1, 0][3]),
                             (11, ef[1, 0][4]),
                             (12, ef[1, 1][0]),
                             (13, ef[1, 1][1]),
                             (13, lambda: pv13["pv"](0)),
                             (14, ef[1, 1][2]),
                             (14, lambda: pv13["pv"](1)),
                             (15, ef[1, 1][3]),
                             (15, lambda: pv13["pv"](2)),
                             (15, lambda: pv13["pv"](3))),
                    expose=pv13)
                fin[1, 3](first_s=4, tail=True)
                ef[1, 1][4]()
                emit_final(1, 3, split=True)

    nc.compile()
    return nc


def _prep(inputs, mode):
    query = np.asarray(inputs["query"], np.float32)
    key = np.asarray(inputs["key"], np.float32)
    value = np.asarray(inputs["value"], np.float32)
    Wq = np.asarray(inputs["Wq"], np.float32)
    Wk = np.asarray(inputs["Wk"], np.float32)
    Wv = np.asarray(inputs["Wv"], np.float32)
    Wo = np.asarray(inputs["Wo"], np.float32)

    xqT = np.ascontiguousarray(query.reshape(T, D).T).astype(BF)
    xkT = np.ascontiguousarray(key.reshape(T, D).T).astype(BF)
    xvT = np.ascontiguousarray(value.reshape(T, D).T).astype(BF)
    woT = np.ascontiguousarray(Wo.T).astype(BF)
    woT_loc = [np.ascontiguousarray(woT[128 * c:128 * (c + 1), :])
               for c in range(N_CORES)]
    w3_loc = []
    for c in range(N_CORES):
        blk = np.concatenate(
            [Wq[EPC * c:EPC * (c + 1), :].T,
             Wk[EPC * c:EPC * (c + 1), :].T,
             Wv[EPC * c:EPC * (c + 1), :].T], axis=1)
        w3_loc.append(np.ascontiguousarray(blk).astype(BF))

    biasT = None
    if mode == "generic":
        m2 = np.asarray(inputs["mask"])[0, 0]
        biasT = np.ascontiguousarray(
            np.where(m2.T == 0, np.float32(-1e9), np.float32(0.0))).astype(BF)

    in_maps = []
    for c in range(N_CORES):
        m = {"xq": xqT, "xk": xkT, "xv": xvT,
             "w3": w3_loc[c], "wo": woT_loc[c]}
        if biasT is not None:
            m["bias"] = biasT
        in_maps.append(m)
    return in_maps


def _mask_mode(mask):
    m2 = np.asarray(mask)[0, 0]
    if (m2 == 1).all():
        return "none"
    if np.array_equal(m2 != 0, np.tril(np.ones(m2.shape, dtype=bool))):
        return "causal"
    return "generic"


def kernel(**inputs) -> np.ndarray:
    mode = _mask_mode(inputs["mask"])
    if mode not in _CACHE:
        _CACHE[mode] = _build(mode)
    nc = _CACHE[mode]
    in_maps = _prep(inputs, mode)
    res = bass_utils.run_bass_kernel_spmd(nc, in_maps,
                                          core_ids=list(range(N_CORES)))
    out = res.results[0]["out"].astype(np.float32)
    for c in range(1, N_CORES):
        out += res.results[c]["out"]
    return out.reshape(B, S, D)


if __name__ == "__main__":
    rng = np.random.default_rng(0)
    inputs = {
        "query": rng.standard_normal((B, S, D)).astype(np.float32),
        "key": rng.standard_normal((B, S, D)).astype(np.float32),
        "value": rng.standard_normal((B, S, D)).astype(np.float32),
        "mask": np.tril(np.ones((S, S), np.int32))[None, None],
        "Wq": (rng.standard_normal((D, D)) / 32).astype(np.float32),
        "Wk": (rng.standard_normal((D, D)) / 32).astype(np.float32),
        "Wv": (rng.standard_normal((D, D)) / 32).astype(np.float32),
        "Wo": (rng.standard_normal((D, D)) / 32).astype(np.float32),
    }
    got = kernel(**inputs)
    print("kernel ran, out shape", got.shape, "finite:", np.isfinite(got).all())
